# revision 125
# baseline (speedup 1.0000x reference)
"""Trainium2 Bass kernel for nn_Attention_44074954391876.

Dense ViT-style attention (B=64, N=257 tokens, D=1024, H=16 heads) with a
gathered relative-position bias, executed data-parallel over batch across
8 NeuronCores (8 items per core).

Per-core pipeline (inputs/weights in bf16, accumulation in fp32 PSUM,
scores in f32r):
  A. load x (bf16), PE-transpose to xT (feature-major)
  B. qkT = Wqk @ xT     (feature-major, q pre-scaled by 1/sqrt(hd) on host)
  C. v   = x @ Wv.T     (token-major, ones column appended per head ->
     denominator row in AV); the last token's v row is computed
     feature-major via 64 free-1 matmuls + an XBAR-transpose flatten
     (PE cost ~0 instead of 8192 cycles)
  D. per head pair: ST = kT.T@qT; P = exp(ST)*exp(B) where exp(B) is the
     host-precomputed exponentiated rel-pos bias (bf16 multiply on
     GPSIMD/DVE - no PE identity-matmul); avT = v.T@P (+denominator row),
     reciprocal (DVE), broadcast via rank-1 ones-matmul, normalize
     (DVE) -> avT bf16; spill avT to DRAM scratch
  E. y = avT.T @ Wp.T + b (token-major), write out fp32

Scheduling: D(i) head-pair chunks are interleaved between item i+1's
B-matmul chunks (and D of the last item between the first E chunks) with a
2-deep scores->AV software pipeline, so every cross-engine
exp/mul/recip/broadcast chain hides behind independent PE work. Weights
stream as a few big-AP DMAs in consumption order.

Softmax uses the identity exp(s)/sum(exp(s)) without max-subtraction: with
the reference's 0.02-scaled weights, |logits| < ~10, far inside fp32 exp
range, so this is numerically safe.
"""

import sys

if "/opt/trn_rl_repo" not in sys.path:
    sys.path.insert(0, "/opt/trn_rl_repo")

import numpy as np
import ml_dtypes

B = 64          # batch
N = 257         # tokens
D = 1024        # model dim
H = 16          # heads
HD = 64         # head dim
NCORES = 8
BL = B // NCORES            # items per core
SCALE = HD ** -0.5
TT = [(0, 128), (128, 128), (256, 1)]   # token tiles (offset, size)
NE = 258                                 # N padded even (fp32r needs even N)
CT = 8                                   # 128-wide channel chunks of D

USE_F32R = True

_CACHE = {}


def _build(R, use_f32r=USE_F32R, reps=1, phases="ABCDE"):
    """Build the SPMD Bass program. R = leading dim of the rel-bias input
    (1 = shared across items; BL = per-item, used when attn_mask is not
    all-ones and the mask bias has been folded into the rel bias).
    reps > 1 repeats the whole pipeline (for differential timing)."""
    import concourse.bass as bass
    import concourse.tile as tile
    from concourse import bacc, mybir

    f32 = mybir.dt.float32
    f32r = mybir.dt.float32r
    bf16 = mybir.dt.bfloat16
    Exp = mybir.ActivationFunctionType.Exp

    nc = bacc.Bacc("TRN2", target_bir_lowering=False, debug=False,
                   num_devices=NCORES)

    x_d = nc.dram_tensor("x", [BL * N, D], bf16, kind="ExternalInput")
    wqk_d = nc.dram_tensor("wqk", [D, 2 * D], bf16, kind="ExternalInput")
    wv_d = nc.dram_tensor("wv", [D, D], bf16, kind="ExternalInput")
    wp_d = nc.dram_tensor("wp", [D, D], bf16, kind="ExternalInput")
    qkb_d = nc.dram_tensor("qkb", [128, 16], f32, kind="ExternalInput")
    vb_d = nc.dram_tensor("vb", [128, D], f32, kind="ExternalInput")
    vbt_d = nc.dram_tensor("vbt", [128, 8], f32, kind="ExternalInput")
    pb_d = nc.dram_tensor("pb", [128, D], f32, kind="ExternalInput")
    relbt_d = nc.dram_tensor("relbt", [R, H, N, N], bf16, kind="ExternalInput")
    ones_d = nc.dram_tensor("ones", [128, 64], f32r, kind="ExternalInput")
    idf_d = nc.dram_tensor("idf", [128, 128], f32r, kind="ExternalInput")
    y_d = nc.dram_tensor("y", [BL * N, D], f32, kind="ExternalOutput")

    from concourse.masks import make_identity

    from contextlib import ExitStack

    with tile.TileContext(nc) as tc, ExitStack() as es:
        if True:
            dpool = es.enter_context(
                tc.tile_pool(name="dram", bufs=1, space="DRAM"))
            avt_sc = dpool.tile([BL, D, N], bf16)
            vls_sc = dpool.tile([BL, 128, 8], bf16)

            if True:
                ep = es.enter_context
                cpool = ep(tc.tile_pool(name="consts", bufs=1))
                xpool = ep(tc.tile_pool(name="xin", bufs=3))
                xtpool = ep(tc.tile_pool(name="xt", bufs=8))
                qktpool = ep(tc.tile_pool(name="qkt", bufs=34))
                vpool = ep(tc.tile_pool(name="v", bufs=4))
                ptpool = ep(tc.tile_pool(name="pt", bufs=10))
                etpool = ep(tc.tile_pool(name="et", bufs=4))
                rdpool = ep(tc.tile_pool(name="rd", bufs=2))
                bcpool = ep(tc.tile_pool(name="bcsb", bufs=4))
                avtpool = ep(tc.tile_pool(name="avt", bufs=2))
                rpool = ep(tc.tile_pool(name="relb", bufs=(1 if R == 1
                                                           else 2)))
                avipool = ep(tc.tile_pool(name="avi", bufs=3))
                ps_a = ep(tc.tile_pool(name="ps_a", bufs=2, space="PSUM"))
                ps_st = ep(tc.tile_pool(name="ps_st", bufs=2, space="PSUM"))
                ps_av = ep(tc.tile_pool(name="ps_av", bufs=2, space="PSUM"))
                def load_x(i, split=False):
                    xins = []
                    for j, (o, sz) in enumerate(TT):
                        xi = xpool.tile([sz, D], bf16,
                                        tag=("x" if sz == 128 else "xs"))
                        if split and j == 0:
                            # halve the very first transfer so item-0's
                            # transposes can start a microsecond earlier
                            nc.sync.dma_start(
                                xi[:, 0:512],
                                x_d[i * N + o:i * N + o + sz, 0:512])
                            nc.sync.dma_start(
                                xi[:, 512:D],
                                x_d[i * N + o:i * N + o + sz, 512:D])
                        else:
                            nc.sync.dma_start(xi[:],
                                              x_d[i * N + o:i * N + o + sz, :])
                        xins.append((xi, o, sz))
                    return xins

                xins_pre = load_x(0, split=True)

                # ---- constants ----
                # one big SBUF tile per weight matrix; DMAs are issued in
                # consumption order (wqk quarters mt-major, then wv) as a few
                # big-AP transfers so the ACT sequencer isn't clogged with
                # descriptor-generation time at startup
                wqkbig = cpool.tile([128, CT * 2 * D], bf16, tag="wqk")
                wvbig = cpool.tile([128, CT * D], bf16, tag="wv")

                def wqk_sl(k, lo, hi):
                    return wqkbig[:, k * 2 * D + lo:k * 2 * D + hi]

                def wv_sl(k, lo, hi):
                    return wvbig[:, k * D + lo:k * D + hi]
                wqk_src = wqk_d.rearrange("(k p) c -> p k c", p=128)
                wqk_dst = wqkbig[:].rearrange("p (k c) -> p k c", c=2 * D)
                wv_src = wv_d.rearrange("(k p) c -> p k c", p=128)
                wv_dst = wvbig[:].rearrange("p (k c) -> p k c", c=D)
                for eighth in range(8):
                    nc.scalar.dma_start(
                        wqk_dst[:, :, eighth * 256:(eighth + 1) * 256],
                        wqk_src[:, :, eighth * 256:(eighth + 1) * 256])
                for half in range(2):
                    nc.scalar.dma_start(
                        wv_dst[:, :, half * 512:(half + 1) * 512],
                        wv_src[:, :, half * 512:(half + 1) * 512])
                qkb = cpool.tile([128, 16], f32, tag="qkb")
                nc.sync.dma_start(qkb[:], qkb_d[:])
                vb = cpool.tile([128, D], f32, tag="vb")
                nc.sync.dma_start(vb[:], vb_d[:])
                vbt = cpool.tile([128, 8], f32, tag="vbt")
                nc.sync.dma_start(vbt[:], vbt_d[:])
                wpbig = cpool.tile([128, CT * D], bf16, tag="wp")
                if "E" in phases:
                    nc.scalar.dma_start(
                        wpbig[:].rearrange("p (k c) -> p k c", c=D),
                        wp_d.rearrange("(k p) c -> p k c", p=128))
                pb = cpool.tile([128, D], f32, tag="vb2")
                nc.scalar.dma_start(pb[:], pb_d[:])
                idf = cpool.tile([128, 128], f32r, tag="idf")
                nc.sync.dma_start(idf[:], idf_d[:])
                idb = cpool.tile([128, 128], bf16, tag="idb")
                make_identity(nc, idb[:])
                ones = cpool.tile([128, 64], f32r, tag="ones")
                nc.sync.dma_start(ones[:], ones_d[:])

                def load_relb(r):
                    # one TILE per (k-chunk, 4-head group) so the first head
                    # pairs of D(0) unblock as soon as their slice lands,
                    # not when the full 16-head transfer completes; DMAs are
                    # issued group-major for the same reason
                    out = [[None] * 4 for _ in TT]
                    for g in range(4):
                        for kc, (ko, ks) in enumerate(TT):
                            t = rpool.tile([ks, 4 * N], bf16,
                                           tag=f"rb{kc}_{g}")
                            nc.sync.dma_start(
                                t[:ks].rearrange("p (h c) -> p h c", c=N),
                                relbt_d[r, 4 * g:4 * g + 4,
                                        ko:ko + ks, :].transpose([1, 0, 2]))
                            out[kc][g] = t
                    return out

                relb0 = load_relb(0) if R == 1 else None

                # D: attention per head pair. The rel-pos bias is folded in
                # as exp(s+b) = exp(s)*exp(b): exp(b) is precomputed on host
                # (item-invariant), applied as a bf16 DVE multiply — no PE
                # identity-matmul needed.
                def scores_pts(qkt, relbI, hp):
                    qt = qkt[hp]
                    kt_t = qkt[8 + hp]
                    pts = []
                    for kc, (ko, ks) in enumerate(TT):
                        st = ps_st.tile([128, 1024], f32, tag="st")
                        for idx in range(2):
                            po = idx * 64
                            fo = idx * 512
                            nc.tensor.matmul(
                                st[:ks, fo:fo + NE],
                                kt_t[po:po + 64, ko:ko + ks],
                                qt[po:po + 64, 0:NE],
                                start=True, stop=True)
                        et = etpool.tile([128, 2 * NE], bf16, tag="et")
                        ein = st[:ks].rearrange(
                            "p (b c) -> p b c", b=2)[:, :, 0:N]
                        emid = et[:ks].rearrange(
                            "p (b c) -> p b c", c=NE)[:, :, 0:N]
                        nc.scalar.activation(emid, ein, Exp)
                        pt = ptpool.tile([128, 2 * NE], bf16, tag="pt")
                        eout = pt[:ks].rearrange(
                            "p (b c) -> p b c", c=NE)[:, :, 0:N]
                        off = (hp % 2) * 2 * N
                        rb = relbI[kc][hp // 2][:ks,
                                                off:off + 2 * N
                                                ].rearrange(
                                                    "p (b c) -> p b c", c=N)
                        # kc0/kc1 bias-multiplies run on the otherwise-idle
                        # GPSIMD engine to keep the DVE off the critical path
                        eng = nc.gpsimd if kc < 2 else nc.vector
                        eng.tensor_mul(eout, emid, rb)
                        pts.append(pt)
                    return pts

                def av_norm(i, hp, pts, vt):
                    avt = avtpool.tile([64, 2 * N], bf16, tag="avt")
                    avs, rds = [], []
                    # both AV accumulations first: AV(h1)'s matmuls cover the
                    # recip(h0) latency so bc(h0) doesn't stall the PE
                    for idx, h in enumerate((2 * hp, 2 * hp + 1)):
                        av = ps_av.tile([128, 512], f32, tag="av")
                        for kc, (ko, ks) in enumerate(TT):
                            nc.tensor.matmul(
                                av[0:65, 0:NE],
                                vt[kc][:, h * 65:(h + 1) * 65],
                                pts[kc][:ks, idx * NE:(idx + 1) * NE],
                                start=(kc == 0), stop=(kc == 2))
                        rd = rdpool.tile([128, NE], f32r, tag="rd")
                        with nc.allow_low_precision(
                                reason="fp32r softmax denom"):
                            nc.vector.reciprocal(rd[64:65, 0:N],
                                                 av[64:65, 0:N])
                        avs.append(av)
                        rds.append(rd)
                    for idx in range(2):
                        bc = ps_st.tile([64, 512], f32, tag="st")
                        nc.tensor.matmul(
                            bc[0:64, 0:NE],
                            ones[64:65, 0:64],
                            rds[idx][64:65, 0:NE],
                            start=True, stop=True)
                        bcsb = bcpool.tile([64, N], f32, tag="bcsb")
                        nc.scalar.copy(bcsb[:], bc[0:64, 0:N])
                        nc.vector.tensor_mul(
                            avt[:, idx * N:(idx + 1) * N],
                            avs[idx][0:64, 0:N], bcsb[:])
                    nc.sync.dma_start(
                        avt_sc[i].rearrange(
                            "(g p) c -> g p c",
                            p=64)[2 * hp:2 * hp + 2, :, :].rearrange(
                                "g p c -> p g c"),
                        avt[:].rearrange("p (g c) -> p g c", c=N))

                state = {}
                pending_hp = []
                PIPE_D = 2

                # two-stage software pipeline within D: scores(hp) is
                # emitted before AV(hp-2) so the PE never waits on exp/mul
                def emit_hp(i, hp):
                    qkt_i, vt_i, relb_i = state[i]
                    pending_hp.append(
                        (i, hp, scores_pts(qkt_i, relb_i, hp), vt_i))
                    if len(pending_hp) > PIPE_D:
                        av_norm(*pending_hp.pop(0))

                def emit_A(i, xins):
                    xts = []
                    for ct in range(CT):
                        ps = ps_a.tile([128, 512], f32, tag="psa")
                        psb = ps[:].bitcast(bf16)
                        for (xi, o, sz) in xins:
                            nc.tensor.transpose(
                                psb[:, o:o + sz],
                                xi[:, ct * 128:(ct + 1) * 128],
                                idb[:sz, :sz])
                        xt = xtpool.tile([128, NE], bf16, tag="xt")
                        nc.vector.tensor_copy(xt[:, 0:N], psb[:, 0:N])
                        xts.append(xt)
                    return xts

                def emit_B(xts, qkt, mts):
                    for mt in mts:
                        ps = ps_a.tile([128, 512], f32, tag="psa")
                        for kt in range(CT):
                            nc.tensor.matmul(
                                ps[:, 0:NE],
                                wqk_sl(kt, mt * 128, (mt + 1) * 128),
                                xts[kt][:, 0:NE],
                                start=(kt == 0), stop=(kt == CT - 1))
                        t = qktpool.tile([128, NE], f32r, tag="qkt")
                        nc.vector.tensor_scalar_add(t[:, 0:N], ps[:, 0:N],
                                                    qkb[:, mt:mt + 1])
                        qkt.append(t)

                def emit_C_last(i, xts, vt):
                    # last token's v row, feature-major: 64 free-1 matmuls
                    # (cost ~0 on PE vs 8192 cycles for a 1-token C tile),
                    # then a tiny flatten-DMA into the [1, H*65] layout the
                    # kc2 AV matmul wants (ones column pre-written)
                    ps = ps_a.tile([128, 512], f32, tag="psa")
                    for vc in range(CT):
                        for kt in range(CT):
                            nc.tensor.matmul(
                                ps[:, vc:vc + 1],
                                wv_sl(kt, vc * 128, (vc + 1) * 128),
                                xts[kt][:, 256:257],
                                start=(kt == 0), stop=(kt == CT - 1))
                    vl8 = bcpool.tile([128, 8], bf16, tag="vl8")
                    nc.vector.tensor_add(vl8[:], ps[:, 0:8], vbt[:])
                    vtile = vpool.tile([1, H * 65], bf16, tag="vs")
                    vdst = vtile[:1].rearrange("p (h c) -> p h c", c=65)
                    nc.vector.tensor_copy(
                        vdst[:, :, 64:65],
                        ones[:1, 0:16].rearrange("p (a b) -> p a b", b=1))
                    nc.sync.dma_start(vls_sc[i], vl8[:])
                    vl8t = bcpool.tile([8, 128], bf16, tag="vl8t")
                    nc.sync.dma_start_transpose(vl8t[:], vls_sc[i])
                    for ph in range(2):
                        dst = vtile[0:1].rearrange(
                            "p (vc r) -> p vc r",
                            r=130)[:, :, ph * 65:ph * 65 + 64]
                        nc.sync.dma_start(
                            dst, vl8t[:, ph * 64:(ph + 1) * 64])
                    vt.append(vtile)

                def emit_C(xts, vt, j):
                    o, sz = TT[j]
                    vtile = vpool.tile([sz, H * 65], bf16,
                                       tag=("v" if sz == 128 else "vs"))
                    vdst = vtile[:sz].rearrange("p (h c) -> p h c", c=65)
                    for ntc in range(2):
                        ps = ps_a.tile([128, 512], f32, tag="psa")
                        for kt in range(CT):
                            nc.tensor.matmul(
                                ps[:sz, :],
                                xts[kt][:, o:o + sz],
                                wv_sl(kt, ntc * 512, (ntc + 1) * 512),
                                start=(kt == 0), stop=(kt == CT - 1))
                        nc.vector.tensor_add(
                            vdst[:, ntc * 8:(ntc + 1) * 8, 0:64],
                            ps[:sz].rearrange("p (h c) -> p h c", c=64),
                            vb[:sz].rearrange(
                                "p (h c) -> p h c",
                                c=64)[:, ntc * 8:(ntc + 1) * 8, :])
                    nc.vector.tensor_copy(
                        vdst[:, :, 64:65],
                        ones[:sz, 0:16].rearrange("p (a b) -> p a b", b=1))
                    vt.append(vtile)

                # ---- per-item phases A-D, software-pipelined ----
                # D(i) head-pair chunks are interleaved between B-matmul
                # chunks of item i+1, so every cross-engine exp/recip/mul
                # chain of D has a couple microseconds of independent PE
                # work in front of it.
                for rep in range(reps):
                  for i in range(BL):
                    relb = relb0 if R == 1 else load_relb(i)
                    xins = xins_pre if (rep == 0 and i == 0) else load_x(i)
                    xts = emit_A(i, xins)
                    qkt, vt = [], []
                    lag = ("D" in phases and i > 0)
                    if "B" in phases:
                        if lag:
                            for hp in range(8):
                                emit_B(xts, qkt, [2 * hp, 2 * hp + 1])
                                emit_hp(i - 1, hp)
                        else:
                            emit_B(xts, qkt, range(16))
                    if "C" in phases:
                        for j in range(2):
                            emit_C(xts, vt, j)
                        emit_C_last(i, xts, vt)
                    if lag:
                        state.pop(i - 1)
                    state[i] = (qkt, vt, relb)
                    if ("D" in phases and i == BL - 1
                            and not ("E" in phases and reps == 1)):
                        # no E phase to interleave the last item's D with
                        for hp in range(8):
                            emit_hp(i, hp)
                        while pending_hp:
                            av_norm(*pending_hp.pop(0))

            # ---- phase E: output projection ----
            # Reuses A-D pool slots (same tags) so the wp/pb prefetch and the
            # first proj matmuls overlap the tail of the attention phase.
                def wp_sl(k, lo, hi):
                    return wpbig[:, k * D + lo:k * D + hi]

                def load_avin(i):
                    t = avipool.tile([128, CT * NE], bf16, tag="avi")
                    nc.sync.dma_start(
                        t[:].rearrange("p (k c) -> p k c", c=NE)[:, :, 0:N],
                        avt_sc[i].rearrange("(k p) c -> p k c", p=128))
                    return t

                def emit_E_chunk(i, avin, mo, ms):
                    ysb = xpool.tile([128, D], f32, tag="x")
                    for ntc in range(2):
                        ps = ps_a.tile([128, 512], f32, tag="psa")
                        for kt in range(CT):
                            nc.tensor.matmul(
                                ps[:ms, :],
                                avin[:, kt * NE + mo:kt * NE + mo + ms],
                                wp_sl(kt, ntc * 512, (ntc + 1) * 512),
                                start=(kt == 0), stop=(kt == CT - 1))
                        nc.vector.tensor_add(
                            ysb[:ms, ntc * 512:(ntc + 1) * 512],
                            ps[:ms, :],
                            pb[:ms, ntc * 512:(ntc + 1) * 512])
                    nc.sync.dma_start(
                        y_d[i * N + mo:i * N + mo + ms, :], ysb[:ms, :])

                for rep in range(reps if "E" in phases else 0):
                    tail_D = ("D" in phases and reps == 1)
                    avins = {0: load_avin(0), 1: load_avin(1)}
                    cno = 0
                    for i in range(BL):
                        avins.setdefault(i, load_avin(i))
                        if i + 1 < BL and cno >= 4:
                            avins.setdefault(i + 1, load_avin(i + 1))
                        for (mo, ms) in TT[:2]:
                            # drip the last item's D head-pairs between the
                            # first E chunks (E(i<7) only needs spills that
                            # are already in DRAM)
                            if tail_D and cno < 8:
                                emit_hp(BL - 1, cno)
                            if tail_D and cno == 8:
                                while pending_hp:
                                    av_norm(*pending_hp.pop(0))
                            emit_E_chunk(i, avins[i], mo, ms)
                            cno += 1
                        if cno == 10:
                            # last token of each item, batched: [128, BL] per
                            # chunk (needs all avt spills incl the last item)
                            avl = cpool.tile([128, CT * BL], bf16, tag="idf")
                            for kt in range(CT):
                                nc.scalar.dma_start(
                                    avl[:, kt * BL:(kt + 1) * BL],
                                    avt_sc[:, kt * 128:(kt + 1) * 128,
                                           N - 1:N].rearrange(
                                               "g p c -> p (g c)"))
                        avins.pop(i)
                    # batched remainder tokens (one per item): [BL, D]
                    ysb = xpool.tile([128, D], f32, tag="x")
                    for ntc in range(2):
                        ps = ps_a.tile([128, 512], f32, tag="psa")
                        for kt in range(CT):
                            nc.tensor.matmul(
                                ps[:BL, :],
                                avl[:, kt * BL:(kt + 1) * BL],
                                wp_sl(kt, ntc * 512, (ntc + 1) * 512),
                                start=(kt == 0), stop=(kt == CT - 1))
                        nc.vector.tensor_add(
                            ysb[:BL, ntc * 512:(ntc + 1) * 512],
                            ps[:BL, :],
                            pb[:BL, ntc * 512:(ntc + 1) * 512])
                    nc.sync.dma_start(
                        y_d.rearrange("(g n) d -> g n d",
                                      n=N)[:, N - 1, :], ysb[:BL, :])

    nc.finalize()
    return nc


def _get_nc(R, reps=1, phases="ABCDE"):
    key = (R, USE_F32R, reps, phases)
    if key not in _CACHE:
        _CACHE[key] = _build(R, reps=reps, phases=phases)
    return _CACHE[key]


def _get_runner(R):
    """Build (once) a persistent jitted SPMD executable for the program."""
    key = ("runner", R, USE_F32R)
    if key in _CACHE:
        return _CACHE[key]
    import jax
    from jax.sharding import Mesh, PartitionSpec, NamedSharding
    from jax.experimental.shard_map import shard_map
    from concourse.bass2jax import (_bass_exec_p, partition_id_tensor,
                                    install_neuronx_cc_hook)
    import concourse.mybir as mybir

    install_neuronx_cc_hook()
    nc = _get_nc(R)
    partition_name = (nc.partition_id_tensor.name
                      if nc.partition_id_tensor else None)
    in_names, out_names, out_avals, out_shapes = [], [], [], []
    for alloc in nc.m.functions[0].allocations:
        if not isinstance(alloc, mybir.MemoryLocationSet):
            continue
        name = alloc.memorylocations[0].name
        if alloc.kind == "ExternalInput":
            if name != partition_name:
                in_names.append(name)
        elif alloc.kind == "ExternalOutput":
            shape = list(alloc.tensor_shape)
            np_dt = mybir.dt.np(alloc.dtype)
            out_avals.append(jax.core.ShapedArray(tuple(shape), np_dt))
            out_names.append(name)
            out_shapes.append((shape, np_dt))
    n_params = len(in_names)
    n_outs = len(out_names)
    in_names_all = (in_names + out_names +
                    ([partition_name] if partition_name else []))

    def _body(*args):
        operands = list(args)
        if partition_name is not None:
            operands.append(partition_id_tensor())
        return tuple(_bass_exec_p.bind(
            *operands, out_avals=tuple(out_avals),
            in_names=tuple(in_names_all), out_names=tuple(out_names),
            lowering_input_output_aliases=(),
            sim_require_finite=True, sim_require_nnan=True, nc=nc))

    devices = jax.devices()[:NCORES]
    mesh = Mesh(np.asarray(devices), ("core",))
    # per-core inputs are sharded over the core axis; shared tensors are
    # replicated (uploaded once, not 8x)
    percore = {"x"} | ({"relbt"} if R != 1 else set())
    in_specs = tuple(PartitionSpec("core") if nm in percore
                     else PartitionSpec() for nm in in_names) + \
        (PartitionSpec("core"),) * n_outs
    sharded = jax.jit(shard_map(
        _body, mesh=mesh, in_specs=in_specs,
        out_specs=(PartitionSpec("core"),) * n_outs, check_rep=False),
        keep_unused=True)
    shard_c = NamedSharding(mesh, PartitionSpec("core"))
    shard_r = NamedSharding(mesh, PartitionSpec())
    _CACHE[key] = (sharded, in_names, out_names, out_shapes,
                   percore, shard_c, shard_r)
    return _CACHE[key]


def kernel(x, qkv_w, q_bias, v_bias, rel_pos_table, proj_w, proj_b,
           rel_pos_index, attn_mask):
    import jax

    bf16 = ml_dtypes.bfloat16
    x = np.ascontiguousarray(np.asarray(x, dtype=np.float32))
    qkv_w = np.asarray(qkv_w, dtype=np.float32)
    q_bias = np.asarray(q_bias, dtype=np.float32)
    v_bias = np.asarray(v_bias, dtype=np.float32)
    rel_pos_table = np.asarray(rel_pos_table, dtype=np.float32)
    proj_w = np.asarray(proj_w, dtype=np.float32)
    proj_b = np.asarray(proj_b, dtype=np.float32)
    rel_pos_index = np.asarray(rel_pos_index)
    attn_mask = np.asarray(attn_mask)

    # host-side prep (sharding + weight layout, no reduction of device work)
    wqk = np.ascontiguousarray(qkv_w[:2 * D].T)          # [D, 2D]
    wqk[:, :D] *= SCALE                                   # fold q scaling
    wqk = wqk.astype(bf16)
    wv = np.ascontiguousarray(qkv_w[2 * D:].T).astype(bf16)  # [D, D]
    wp = np.ascontiguousarray(proj_w.T).astype(bf16)      # [D, D]
    qkb = np.concatenate([q_bias * SCALE,
                          np.zeros(D, np.float32)]).astype(np.float32)
    qkb_p = np.ascontiguousarray(qkb.reshape(16, 128).T)  # [128, 16]
    vb = np.ascontiguousarray(np.broadcast_to(v_bias, (128, D)))
    vbt = np.ascontiguousarray(v_bias.reshape(8, 128).T)  # [128, 8]
    pb = np.ascontiguousarray(np.broadcast_to(proj_b, (128, D)))

    # gathered relative-position bias, pre-transposed to [H, k, q] and
    # EXPONENTIATED on host: device applies it as exp(s)*exp(b)
    relbT = np.ascontiguousarray(
        rel_pos_table[rel_pos_index].transpose(2, 1, 0))  # [H, N(k), N(q)]

    mask_all = bool(attn_mask.all())
    if mask_all:
        R = 1
        relbt_per_core = [np.exp(relbT)[None].astype(bf16)] * NCORES
    else:
        R = BL
        # masked keys get exp(b-60) ~ 1e-26: negligible in the softmax sum
        mb = np.where(attn_mask, np.float32(0),
                      np.float32(-60.0)).astype(np.float32)  # [B, N] over k
        relbt_per_core = []
        for c in range(NCORES):
            m = mb[c * BL:(c + 1) * BL]            # [BL, N]
            t = np.exp(relbT[None] + m[:, None, :, None])
            relbt_per_core.append(t.astype(bf16))

    in_maps = []
    for c in range(NCORES):
        in_maps.append({
            "x": np.ascontiguousarray(
                x[c * BL:(c + 1) * BL].reshape(BL * N, D)).astype(bf16),
            "wqk": wqk, "wv": wv, "wp": wp,
            "qkb": qkb_p, "vb": vb, "vbt": vbt, "pb": pb,
            "ones": np.ones((128, 64), np.float32),
            "idf": np.eye(128, dtype=np.float32),
            "relbt": relbt_per_core[c],
        })

    (sharded, in_names, out_names, out_shapes,
     percore, shard_c, shard_r) = _get_runner(R)
    host_in, shardings = [], []
    for nm in in_names:
        if nm in percore:
            host_in.append(np.concatenate(
                [np.asarray(in_maps[c][nm]) for c in range(NCORES)], axis=0))
            shardings.append(shard_c)
        else:
            host_in.append(np.asarray(in_maps[0][nm]))
            shardings.append(shard_r)
    for (s, dt) in out_shapes:
        host_in.append(np.zeros((NCORES * s[0], *s[1:]), dt))
        shardings.append(shard_c)
    dev_in = jax.device_put(host_in, shardings)
    out = sharded(*dev_in)
    yi = out_names.index("y")
    y = np.asarray(out[yi]).reshape(NCORES, BL, N, D).reshape(B, N, D)
    return np.ascontiguousarray(y.astype(np.float32))



# revision 126
# speedup vs baseline: 1.0018x; 1.0018x over previous
"""Trainium2 Bass kernel for nn_Attention_44074954391876.

Dense ViT-style attention (B=64, N=257 tokens, D=1024, H=16 heads) with a
gathered relative-position bias, executed data-parallel over batch across
8 NeuronCores (8 items per core).

Per-core pipeline (inputs/weights in bf16, accumulation in fp32 PSUM,
scores in f32r):
  A. load x (bf16), PE-transpose to xT (feature-major)
  B. qkT = Wqk @ xT     (feature-major, q pre-scaled by 1/sqrt(hd) on host)
  C. v   = x @ Wv.T     (token-major, ones column appended per head ->
     denominator row in AV); the last token's v row is computed
     feature-major via 64 free-1 matmuls + an XBAR-transpose flatten
     (PE cost ~0 instead of 8192 cycles)
  D. per head pair: ST = kT.T@qT; P = exp(ST)*exp(B) where exp(B) is the
     host-precomputed exponentiated rel-pos bias (bf16 multiply on
     GPSIMD/DVE - no PE identity-matmul); avT = v.T@P (+denominator row),
     reciprocal (DVE), broadcast via rank-1 ones-matmul, normalize
     (DVE) -> avT bf16; spill avT to DRAM scratch
  E. y = avT.T @ Wp.T + b (token-major), write out fp32

Scheduling: D(i) head-pair chunks are interleaved between item i+1's
B-matmul chunks (and D of the last item between the first E chunks) with a
2-deep scores->AV software pipeline, so every cross-engine
exp/mul/recip/broadcast chain hides behind independent PE work. Weights
stream as a few big-AP DMAs in consumption order.

Softmax uses the identity exp(s)/sum(exp(s)) without max-subtraction: with
the reference's 0.02-scaled weights, |logits| < ~10, far inside fp32 exp
range, so this is numerically safe.
"""

import sys

if "/opt/trn_rl_repo" not in sys.path:
    sys.path.insert(0, "/opt/trn_rl_repo")

import numpy as np
import ml_dtypes

B = 64          # batch
N = 257         # tokens
D = 1024        # model dim
H = 16          # heads
HD = 64         # head dim
NCORES = 8
BL = B // NCORES            # items per core
SCALE = HD ** -0.5
TT = [(0, 128), (128, 128), (256, 1)]   # token tiles (offset, size)
NE = 258                                 # N padded even (fp32r needs even N)
CT = 8                                   # 128-wide channel chunks of D

USE_F32R = True

_CACHE = {}


def _build(R, use_f32r=USE_F32R, reps=1, phases="ABCDE"):
    """Build the SPMD Bass program. R = leading dim of the rel-bias input
    (1 = shared across items; BL = per-item, used when attn_mask is not
    all-ones and the mask bias has been folded into the rel bias).
    reps > 1 repeats the whole pipeline (for differential timing)."""
    import concourse.bass as bass
    import concourse.tile as tile
    from concourse import bacc, mybir

    f32 = mybir.dt.float32
    f32r = mybir.dt.float32r
    bf16 = mybir.dt.bfloat16
    Exp = mybir.ActivationFunctionType.Exp

    nc = bacc.Bacc("TRN2", target_bir_lowering=False, debug=False,
                   num_devices=NCORES)

    x_d = nc.dram_tensor("x", [BL * N, D], bf16, kind="ExternalInput")
    wqk_d = nc.dram_tensor("wqk", [D, 2 * D], bf16, kind="ExternalInput")
    wv_d = nc.dram_tensor("wv", [D, D], bf16, kind="ExternalInput")
    wp_d = nc.dram_tensor("wp", [D, D], bf16, kind="ExternalInput")
    qkb_d = nc.dram_tensor("qkb", [128, 16], f32, kind="ExternalInput")
    vb_d = nc.dram_tensor("vb", [128, D], f32, kind="ExternalInput")
    vbt_d = nc.dram_tensor("vbt", [128, 8], f32, kind="ExternalInput")
    pb_d = nc.dram_tensor("pb", [128, D], f32, kind="ExternalInput")
    relbt_d = nc.dram_tensor("relbt", [R, H, N, N], bf16, kind="ExternalInput")
    ones_d = nc.dram_tensor("ones", [128, 64], f32r, kind="ExternalInput")
    idf_d = nc.dram_tensor("idf", [128, 128], f32r, kind="ExternalInput")
    y_d = nc.dram_tensor("y", [BL * N, D], f32, kind="ExternalOutput")

    from concourse.masks import make_identity

    from contextlib import ExitStack

    with tile.TileContext(nc) as tc, ExitStack() as es:
        if True:
            dpool = es.enter_context(
                tc.tile_pool(name="dram", bufs=1, space="DRAM"))
            avt_sc = dpool.tile([BL, D, N], bf16)
            vls_sc = dpool.tile([BL, 128, 8], bf16)

            if True:
                ep = es.enter_context
                cpool = ep(tc.tile_pool(name="consts", bufs=1))
                xpool = ep(tc.tile_pool(name="xin", bufs=3))
                xtpool = ep(tc.tile_pool(name="xt", bufs=8))
                qktpool = ep(tc.tile_pool(name="qkt", bufs=34))
                vpool = ep(tc.tile_pool(name="v", bufs=4))
                ptpool = ep(tc.tile_pool(name="pt", bufs=10))
                etpool = ep(tc.tile_pool(name="et", bufs=5))
                rdpool = ep(tc.tile_pool(name="rd", bufs=2))
                bcpool = ep(tc.tile_pool(name="bcsb", bufs=4))
                avtpool = ep(tc.tile_pool(name="avt", bufs=2))
                rpool = ep(tc.tile_pool(name="relb", bufs=(1 if R == 1
                                                           else 2)))
                avipool = ep(tc.tile_pool(name="avi", bufs=3))
                ps_a = ep(tc.tile_pool(name="ps_a", bufs=2, space="PSUM"))
                ps_st = ep(tc.tile_pool(name="ps_st", bufs=2, space="PSUM"))
                ps_av = ep(tc.tile_pool(name="ps_av", bufs=2, space="PSUM"))
                def load_x(i, split=False):
                    xins = []
                    for j, (o, sz) in enumerate(TT):
                        xi = xpool.tile([sz, D], bf16,
                                        tag=("x" if sz == 128 else "xs"))
                        if split and j == 0:
                            # halve the very first transfer so item-0's
                            # transposes can start a microsecond earlier
                            nc.sync.dma_start(
                                xi[:, 0:512],
                                x_d[i * N + o:i * N + o + sz, 0:512])
                            nc.sync.dma_start(
                                xi[:, 512:D],
                                x_d[i * N + o:i * N + o + sz, 512:D])
                        else:
                            nc.sync.dma_start(xi[:],
                                              x_d[i * N + o:i * N + o + sz, :])
                        xins.append((xi, o, sz))
                    return xins

                xins_pre = load_x(0, split=True)

                # ---- constants ----
                # one big SBUF tile per weight matrix; DMAs are issued in
                # consumption order (wqk quarters mt-major, then wv) as a few
                # big-AP transfers so the ACT sequencer isn't clogged with
                # descriptor-generation time at startup
                wqkbig = cpool.tile([128, CT * 2 * D], bf16, tag="wqk")
                wvbig = cpool.tile([128, CT * D], bf16, tag="wv")

                def wqk_sl(k, lo, hi):
                    return wqkbig[:, k * 2 * D + lo:k * 2 * D + hi]

                def wv_sl(k, lo, hi):
                    return wvbig[:, k * D + lo:k * D + hi]
                wqk_src = wqk_d.rearrange("(k p) c -> p k c", p=128)
                wqk_dst = wqkbig[:].rearrange("p (k c) -> p k c", c=2 * D)
                wv_src = wv_d.rearrange("(k p) c -> p k c", p=128)
                wv_dst = wvbig[:].rearrange("p (k c) -> p k c", c=D)
                for eighth in range(8):
                    nc.scalar.dma_start(
                        wqk_dst[:, :, eighth * 256:(eighth + 1) * 256],
                        wqk_src[:, :, eighth * 256:(eighth + 1) * 256])
                for half in range(2):
                    nc.scalar.dma_start(
                        wv_dst[:, :, half * 512:(half + 1) * 512],
                        wv_src[:, :, half * 512:(half + 1) * 512])
                qkb = cpool.tile([128, 16], f32, tag="qkb")
                nc.sync.dma_start(qkb[:], qkb_d[:])
                vb = cpool.tile([128, D], f32, tag="vb")
                nc.sync.dma_start(vb[:], vb_d[:])
                vbt = cpool.tile([128, 8], f32, tag="vbt")
                nc.sync.dma_start(vbt[:], vbt_d[:])
                wpbig = cpool.tile([128, CT * D], bf16, tag="wp")
                if "E" in phases:
                    nc.scalar.dma_start(
                        wpbig[:].rearrange("p (k c) -> p k c", c=D),
                        wp_d.rearrange("(k p) c -> p k c", p=128))
                pb = cpool.tile([128, D], f32, tag="vb2")
                nc.scalar.dma_start(pb[:], pb_d[:])
                idf = cpool.tile([128, 128], f32r, tag="idf")
                nc.sync.dma_start(idf[:], idf_d[:])
                idb = cpool.tile([128, 128], bf16, tag="idb")
                make_identity(nc, idb[:])
                ones = cpool.tile([128, 64], f32r, tag="ones")
                nc.sync.dma_start(ones[:], ones_d[:])

                def load_relb(r):
                    # one DMA per k-chunk covering all 16 heads: [ks, H, 257]
                    out = []
                    for kc, (ko, ks) in enumerate(TT):
                        t = rpool.tile([ks, H * N], bf16, tag=f"rb{kc}")
                        nc.sync.dma_start(
                            t[:ks].rearrange("p (h c) -> p h c", c=N),
                            relbt_d[r, :, ko:ko + ks, :].transpose([1, 0, 2]))
                        out.append(t)
                    return out

                relb0 = load_relb(0) if R == 1 else None

                # D: attention per head pair. The rel-pos bias is folded in
                # as exp(s+b) = exp(s)*exp(b): exp(b) is precomputed on host
                # (item-invariant), applied as a bf16 DVE multiply — no PE
                # identity-matmul needed.
                def scores_pts(qkt, relbI, hp):
                    qt = qkt[hp]
                    kt_t = qkt[8 + hp]
                    pts = []
                    for kc, (ko, ks) in enumerate(TT):
                        st = ps_st.tile([128, 1024], f32, tag="st")
                        for idx in range(2):
                            po = idx * 64
                            fo = idx * 512
                            nc.tensor.matmul(
                                st[:ks, fo:fo + NE],
                                kt_t[po:po + 64, ko:ko + ks],
                                qt[po:po + 64, 0:NE],
                                start=True, stop=True)
                        et = etpool.tile([128, 2 * NE], bf16, tag="et")
                        ein = st[:ks].rearrange(
                            "p (b c) -> p b c", b=2)[:, :, 0:N]
                        emid = et[:ks].rearrange(
                            "p (b c) -> p b c", c=NE)[:, :, 0:N]
                        nc.scalar.activation(emid, ein, Exp)
                        pt = ptpool.tile([128, 2 * NE], bf16, tag="pt")
                        eout = pt[:ks].rearrange(
                            "p (b c) -> p b c", c=NE)[:, :, 0:N]
                        rb = relbI[kc][:ks,
                                       2 * hp * N:(2 * hp + 2) * N
                                       ].rearrange("p (b c) -> p b c", c=N)
                        # kc0/kc1 bias-multiplies run on the otherwise-idle
                        # GPSIMD engine to keep the DVE off the critical path
                        eng = nc.gpsimd if kc < 2 else nc.vector
                        eng.tensor_mul(eout, emid, rb)
                        pts.append(pt)
                    return pts

                def av_norm(i, hp, pts, vt):
                    avt = avtpool.tile([64, 2 * N], bf16, tag="avt")
                    avs, rds = [], []
                    # both AV accumulations first: AV(h1)'s matmuls cover the
                    # recip(h0) latency so bc(h0) doesn't stall the PE
                    for idx, h in enumerate((2 * hp, 2 * hp + 1)):
                        av = ps_av.tile([128, 512], f32, tag="av")
                        for kc, (ko, ks) in enumerate(TT):
                            nc.tensor.matmul(
                                av[0:65, 0:NE],
                                vt[kc][:, h * 65:(h + 1) * 65],
                                pts[kc][:ks, idx * NE:(idx + 1) * NE],
                                start=(kc == 0), stop=(kc == 2))
                        rd = rdpool.tile([128, NE], f32r, tag="rd")
                        with nc.allow_low_precision(
                                reason="fp32r softmax denom"):
                            nc.vector.reciprocal(rd[64:65, 0:N],
                                                 av[64:65, 0:N])
                        avs.append(av)
                        rds.append(rd)
                    for idx in range(2):
                        bc = ps_st.tile([64, 512], f32, tag="st")
                        nc.tensor.matmul(
                            bc[0:64, 0:NE],
                            ones[64:65, 0:64],
                            rds[idx][64:65, 0:NE],
                            start=True, stop=True)
                        bcsb = bcpool.tile([64, N], f32, tag="bcsb")
                        nc.scalar.copy(bcsb[:], bc[0:64, 0:N])
                        nc.vector.tensor_mul(
                            avt[:, idx * N:(idx + 1) * N],
                            avs[idx][0:64, 0:N], bcsb[:])
                    nc.sync.dma_start(
                        avt_sc[i].rearrange(
                            "(g p) c -> g p c",
                            p=64)[2 * hp:2 * hp + 2, :, :].rearrange(
                                "g p c -> p g c"),
                        avt[:].rearrange("p (g c) -> p g c", c=N))

                state = {}
                pending_hp = []
                PIPE_D = 2

                # two-stage software pipeline within D: scores(hp) is
                # emitted before AV(hp-2) so the PE never waits on exp/mul
                def emit_hp(i, hp):
                    qkt_i, vt_i, relb_i = state[i]
                    pending_hp.append(
                        (i, hp, scores_pts(qkt_i, relb_i, hp), vt_i))
                    if len(pending_hp) > PIPE_D:
                        av_norm(*pending_hp.pop(0))

                def emit_A(i, xins):
                    xts = []
                    for ct in range(CT):
                        ps = ps_a.tile([128, 512], f32, tag="psa")
                        psb = ps[:].bitcast(bf16)
                        for (xi, o, sz) in xins:
                            nc.tensor.transpose(
                                psb[:, o:o + sz],
                                xi[:, ct * 128:(ct + 1) * 128],
                                idb[:sz, :sz])
                        xt = xtpool.tile([128, NE], bf16, tag="xt")
                        nc.vector.tensor_copy(xt[:, 0:N], psb[:, 0:N])
                        xts.append(xt)
                    return xts

                def emit_B(xts, qkt, mts):
                    for mt in mts:
                        ps = ps_a.tile([128, 512], f32, tag="psa")
                        for kt in range(CT):
                            nc.tensor.matmul(
                                ps[:, 0:NE],
                                wqk_sl(kt, mt * 128, (mt + 1) * 128),
                                xts[kt][:, 0:NE],
                                start=(kt == 0), stop=(kt == CT - 1))
                        t = qktpool.tile([128, NE], f32r, tag="qkt")
                        nc.vector.tensor_scalar_add(t[:, 0:N], ps[:, 0:N],
                                                    qkb[:, mt:mt + 1])
                        qkt.append(t)

                def emit_C_last(i, xts, vt):
                    # last token's v row, feature-major: 64 free-1 matmuls
                    # (cost ~0 on PE vs 8192 cycles for a 1-token C tile),
                    # then a tiny flatten-DMA into the [1, H*65] layout the
                    # kc2 AV matmul wants (ones column pre-written)
                    ps = ps_a.tile([128, 512], f32, tag="psa")
                    for vc in range(CT):
                        for kt in range(CT):
                            nc.tensor.matmul(
                                ps[:, vc:vc + 1],
                                wv_sl(kt, vc * 128, (vc + 1) * 128),
                                xts[kt][:, 256:257],
                                start=(kt == 0), stop=(kt == CT - 1))
                    vl8 = bcpool.tile([128, 8], bf16, tag="vl8")
                    nc.vector.tensor_add(vl8[:], ps[:, 0:8], vbt[:])
                    vtile = vpool.tile([1, H * 65], bf16, tag="vs")
                    vdst = vtile[:1].rearrange("p (h c) -> p h c", c=65)
                    nc.vector.tensor_copy(
                        vdst[:, :, 64:65],
                        ones[:1, 0:16].rearrange("p (a b) -> p a b", b=1))
                    nc.sync.dma_start(vls_sc[i], vl8[:])
                    vl8t = bcpool.tile([8, 128], bf16, tag="vl8t")
                    nc.sync.dma_start_transpose(vl8t[:], vls_sc[i])
                    for ph in range(2):
                        dst = vtile[0:1].rearrange(
                            "p (vc r) -> p vc r",
                            r=130)[:, :, ph * 65:ph * 65 + 64]
                        nc.sync.dma_start(
                            dst, vl8t[:, ph * 64:(ph + 1) * 64])
                    vt.append(vtile)

                def emit_C(xts, vt, j):
                    o, sz = TT[j]
                    vtile = vpool.tile([sz, H * 65], bf16,
                                       tag=("v" if sz == 128 else "vs"))
                    vdst = vtile[:sz].rearrange("p (h c) -> p h c", c=65)
                    for ntc in range(2):
                        ps = ps_a.tile([128, 512], f32, tag="psa")
                        for kt in range(CT):
                            nc.tensor.matmul(
                                ps[:sz, :],
                                xts[kt][:, o:o + sz],
                                wv_sl(kt, ntc * 512, (ntc + 1) * 512),
                                start=(kt == 0), stop=(kt == CT - 1))
                        nc.vector.tensor_add(
                            vdst[:, ntc * 8:(ntc + 1) * 8, 0:64],
                            ps[:sz].rearrange("p (h c) -> p h c", c=64),
                            vb[:sz].rearrange(
                                "p (h c) -> p h c",
                                c=64)[:, ntc * 8:(ntc + 1) * 8, :])
                    nc.vector.tensor_copy(
                        vdst[:, :, 64:65],
                        ones[:sz, 0:16].rearrange("p (a b) -> p a b", b=1))
                    vt.append(vtile)

                # ---- per-item phases A-D, software-pipelined ----
                # D(i) head-pair chunks are interleaved between B-matmul
                # chunks of item i+1, so every cross-engine exp/recip/mul
                # chain of D has a couple microseconds of independent PE
                # work in front of it.
                for rep in range(reps):
                  for i in range(BL):
                    relb = relb0 if R == 1 else load_relb(i)
                    xins = xins_pre if (rep == 0 and i == 0) else load_x(i)
                    xts = emit_A(i, xins)
                    qkt, vt = [], []
                    lag = ("D" in phases and i > 0)
                    if "B" in phases:
                        if lag:
                            for hp in range(8):
                                emit_B(xts, qkt, [2 * hp, 2 * hp + 1])
                                emit_hp(i - 1, hp)
                        else:
                            emit_B(xts, qkt, range(16))
                    if "C" in phases:
                        for j in range(2):
                            emit_C(xts, vt, j)
                        emit_C_last(i, xts, vt)
                    if lag:
                        state.pop(i - 1)
                    state[i] = (qkt, vt, relb)
                    if ("D" in phases and i == BL - 1
                            and not ("E" in phases and reps == 1)):
                        # no E phase to interleave the last item's D with
                        for hp in range(8):
                            emit_hp(i, hp)
                        while pending_hp:
                            av_norm(*pending_hp.pop(0))

            # ---- phase E: output projection ----
            # Reuses A-D pool slots (same tags) so the wp/pb prefetch and the
            # first proj matmuls overlap the tail of the attention phase.
                def wp_sl(k, lo, hi):
                    return wpbig[:, k * D + lo:k * D + hi]

                def load_avin(i):
                    t = avipool.tile([128, CT * NE], bf16, tag="avi")
                    nc.sync.dma_start(
                        t[:].rearrange("p (k c) -> p k c", c=NE)[:, :, 0:N],
                        avt_sc[i].rearrange("(k p) c -> p k c", p=128))
                    return t

                def emit_E_chunk(i, avin, mo, ms):
                    ysb = xpool.tile([128, D], f32, tag="x")
                    for ntc in range(2):
                        ps = ps_a.tile([128, 512], f32, tag="psa")
                        for kt in range(CT):
                            nc.tensor.matmul(
                                ps[:ms, :],
                                avin[:, kt * NE + mo:kt * NE + mo + ms],
                                wp_sl(kt, ntc * 512, (ntc + 1) * 512),
                                start=(kt == 0), stop=(kt == CT - 1))
                        nc.vector.tensor_add(
                            ysb[:ms, ntc * 512:(ntc + 1) * 512],
                            ps[:ms, :],
                            pb[:ms, ntc * 512:(ntc + 1) * 512])
                    nc.sync.dma_start(
                        y_d[i * N + mo:i * N + mo + ms, :], ysb[:ms, :])

                for rep in range(reps if "E" in phases else 0):
                    tail_D = ("D" in phases and reps == 1)
                    avins = {0: load_avin(0), 1: load_avin(1)}
                    cno = 0
                    for i in range(BL):
                        avins.setdefault(i, load_avin(i))
                        if i + 1 < BL and cno >= 4:
                            avins.setdefault(i + 1, load_avin(i + 1))
                        for (mo, ms) in TT[:2]:
                            # drip the last item's D head-pairs between the
                            # first E chunks (E(i<7) only needs spills that
                            # are already in DRAM)
                            if tail_D and cno < 8:
                                emit_hp(BL - 1, cno)
                            if tail_D and cno == 8:
                                while pending_hp:
                                    av_norm(*pending_hp.pop(0))
                            emit_E_chunk(i, avins[i], mo, ms)
                            cno += 1
                        if cno == 10:
                            # last token of each item, batched: [128, BL] per
                            # chunk (needs all avt spills incl the last item)
                            avl = cpool.tile([128, CT * BL], bf16, tag="idf")
                            for kt in range(CT):
                                nc.scalar.dma_start(
                                    avl[:, kt * BL:(kt + 1) * BL],
                                    avt_sc[:, kt * 128:(kt + 1) * 128,
                                           N - 1:N].rearrange(
                                               "g p c -> p (g c)"))
                        avins.pop(i)
                    # batched remainder tokens (one per item): [BL, D]
                    ysb = xpool.tile([128, D], f32, tag="x")
                    for ntc in range(2):
                        ps = ps_a.tile([128, 512], f32, tag="psa")
                        for kt in range(CT):
                            nc.tensor.matmul(
                                ps[:BL, :],
                                avl[:, kt * BL:(kt + 1) * BL],
                                wp_sl(kt, ntc * 512, (ntc + 1) * 512),
                                start=(kt == 0), stop=(kt == CT - 1))
                        nc.vector.tensor_add(
                            ysb[:BL, ntc * 512:(ntc + 1) * 512],
                            ps[:BL, :],
                            pb[:BL, ntc * 512:(ntc + 1) * 512])
                    nc.sync.dma_start(
                        y_d.rearrange("(g n) d -> g n d",
                                      n=N)[:, N - 1, :], ysb[:BL, :])

    nc.finalize()
    return nc


def _get_nc(R, reps=1, phases="ABCDE"):
    key = (R, USE_F32R, reps, phases)
    if key not in _CACHE:
        _CACHE[key] = _build(R, reps=reps, phases=phases)
    return _CACHE[key]


def _get_runner(R):
    """Build (once) a persistent jitted SPMD executable for the program."""
    key = ("runner", R, USE_F32R)
    if key in _CACHE:
        return _CACHE[key]
    import jax
    from jax.sharding import Mesh, PartitionSpec, NamedSharding
    from jax.experimental.shard_map import shard_map
    from concourse.bass2jax import (_bass_exec_p, partition_id_tensor,
                                    install_neuronx_cc_hook)
    import concourse.mybir as mybir

    install_neuronx_cc_hook()
    nc = _get_nc(R)
    partition_name = (nc.partition_id_tensor.name
                      if nc.partition_id_tensor else None)
    in_names, out_names, out_avals, out_shapes = [], [], [], []
    for alloc in nc.m.functions[0].allocations:
        if not isinstance(alloc, mybir.MemoryLocationSet):
            continue
        name = alloc.memorylocations[0].name
        if alloc.kind == "ExternalInput":
            if name != partition_name:
                in_names.append(name)
        elif alloc.kind == "ExternalOutput":
            shape = list(alloc.tensor_shape)
            np_dt = mybir.dt.np(alloc.dtype)
            out_avals.append(jax.core.ShapedArray(tuple(shape), np_dt))
            out_names.append(name)
            out_shapes.append((shape, np_dt))
    n_params = len(in_names)
    n_outs = len(out_names)
    in_names_all = (in_names + out_names +
                    ([partition_name] if partition_name else []))

    def _body(*args):
        operands = list(args)
        if partition_name is not None:
            operands.append(partition_id_tensor())
        return tuple(_bass_exec_p.bind(
            *operands, out_avals=tuple(out_avals),
            in_names=tuple(in_names_all), out_names=tuple(out_names),
            lowering_input_output_aliases=(),
            sim_require_finite=True, sim_require_nnan=True, nc=nc))

    devices = jax.devices()[:NCORES]
    mesh = Mesh(np.asarray(devices), ("core",))
    # per-core inputs are sharded over the core axis; shared tensors are
    # replicated (uploaded once, not 8x)
    percore = {"x"} | ({"relbt"} if R != 1 else set())
    in_specs = tuple(PartitionSpec("core") if nm in percore
                     else PartitionSpec() for nm in in_names) + \
        (PartitionSpec("core"),) * n_outs
    sharded = jax.jit(shard_map(
        _body, mesh=mesh, in_specs=in_specs,
        out_specs=(PartitionSpec("core"),) * n_outs, check_rep=False),
        keep_unused=True)
    shard_c = NamedSharding(mesh, PartitionSpec("core"))
    shard_r = NamedSharding(mesh, PartitionSpec())
    _CACHE[key] = (sharded, in_names, out_names, out_shapes,
                   percore, shard_c, shard_r)
    return _CACHE[key]


def kernel(x, qkv_w, q_bias, v_bias, rel_pos_table, proj_w, proj_b,
           rel_pos_index, attn_mask):
    import jax

    bf16 = ml_dtypes.bfloat16
    x = np.ascontiguousarray(np.asarray(x, dtype=np.float32))
    qkv_w = np.asarray(qkv_w, dtype=np.float32)
    q_bias = np.asarray(q_bias, dtype=np.float32)
    v_bias = np.asarray(v_bias, dtype=np.float32)
    rel_pos_table = np.asarray(rel_pos_table, dtype=np.float32)
    proj_w = np.asarray(proj_w, dtype=np.float32)
    proj_b = np.asarray(proj_b, dtype=np.float32)
    rel_pos_index = np.asarray(rel_pos_index)
    attn_mask = np.asarray(attn_mask)

    # host-side prep (sharding + weight layout, no reduction of device work)
    wqk = np.ascontiguousarray(qkv_w[:2 * D].T)          # [D, 2D]
    wqk[:, :D] *= SCALE                                   # fold q scaling
    wqk = wqk.astype(bf16)
    wv = np.ascontiguousarray(qkv_w[2 * D:].T).astype(bf16)  # [D, D]
    wp = np.ascontiguousarray(proj_w.T).astype(bf16)      # [D, D]
    qkb = np.concatenate([q_bias * SCALE,
                          np.zeros(D, np.float32)]).astype(np.float32)
    qkb_p = np.ascontiguousarray(qkb.reshape(16, 128).T)  # [128, 16]
    vb = np.ascontiguousarray(np.broadcast_to(v_bias, (128, D)))
    vbt = np.ascontiguousarray(v_bias.reshape(8, 128).T)  # [128, 8]
    pb = np.ascontiguousarray(np.broadcast_to(proj_b, (128, D)))

    # gathered relative-position bias, pre-transposed to [H, k, q] and
    # EXPONENTIATED on host: device applies it as exp(s)*exp(b)
    relbT = np.ascontiguousarray(
        rel_pos_table[rel_pos_index].transpose(2, 1, 0))  # [H, N(k), N(q)]

    mask_all = bool(attn_mask.all())
    if mask_all:
        R = 1
        relbt_per_core = [np.exp(relbT)[None].astype(bf16)] * NCORES
    else:
        R = BL
        # masked keys get exp(b-60) ~ 1e-26: negligible in the softmax sum
        mb = np.where(attn_mask, np.float32(0),
                      np.float32(-60.0)).astype(np.float32)  # [B, N] over k
        relbt_per_core = []
        for c in range(NCORES):
            m = mb[c * BL:(c + 1) * BL]            # [BL, N]
            t = np.exp(relbT[None] + m[:, None, :, None])
            relbt_per_core.append(t.astype(bf16))

    in_maps = []
    for c in range(NCORES):
        in_maps.append({
            "x": np.ascontiguousarray(
                x[c * BL:(c + 1) * BL].reshape(BL * N, D)).astype(bf16),
            "wqk": wqk, "wv": wv, "wp": wp,
            "qkb": qkb_p, "vb": vb, "vbt": vbt, "pb": pb,
            "ones": np.ones((128, 64), np.float32),
            "idf": np.eye(128, dtype=np.float32),
            "relbt": relbt_per_core[c],
        })

    (sharded, in_names, out_names, out_shapes,
     percore, shard_c, shard_r) = _get_runner(R)
    host_in, shardings = [], []
    for nm in in_names:
        if nm in percore:
            host_in.append(np.concatenate(
                [np.asarray(in_maps[c][nm]) for c in range(NCORES)], axis=0))
            shardings.append(shard_c)
        else:
            host_in.append(np.asarray(in_maps[0][nm]))
            shardings.append(shard_r)
    for (s, dt) in out_shapes:
        host_in.append(np.zeros((NCORES * s[0], *s[1:]), dt))
        shardings.append(shard_c)
    dev_in = jax.device_put(host_in, shardings)
    out = sharded(*dev_in)
    yi = out_names.index("y")
    y = np.asarray(out[yi]).reshape(NCORES, BL, N, D).reshape(B, N, D)
    return np.ascontiguousarray(y.astype(np.float32))

